# revision 10
# baseline (speedup 1.0000x reference)
import numpy as np

import concourse.bass as bass
import concourse.tile as tile
from concourse import bacc, mybir
from concourse.bass_utils import run_bass_kernel_spmd
from concourse.masks import make_identity

F32 = mybir.dt.float32
F32R = mybir.dt.float32r
BF16 = mybir.dt.bfloat16
AL = mybir.AluOpType
AF = mybir.ActivationFunctionType

P = 128
N_FULL = 16384
N_HALF = 8192
C = 256
HD = 512
OUT = 256
EPS = 1e-5
N_CHUNKS = 32
MY_CHUNKS = 16
INV_N = 1.0 / float(N_FULL)


def build_nc():
    nc = bacc.Bacc(
        "TRN2",
        target_bir_lowering=False,
        debug=False,
        num_devices=8,
    )
    u_d = nc.dram_tensor("u", [N_FULL, C], F32, kind="ExternalInput").ap()
    wq_d = nc.dram_tensor("wq", [HD, C], F32, kind="ExternalInput").ap()
    wk_d = nc.dram_tensor("wk", [HD, C], F32, kind="ExternalInput").ap()
    wv_d = nc.dram_tensor("wv", [HD, C], F32, kind="ExternalInput").ap()
    wo_d = nc.dram_tensor("wo", [OUT, HD], F32, kind="ExternalInput").ap()
    bo_d = nc.dram_tensor("bo", [1, OUT], F32, kind="ExternalInput").ap()
    out_d = nc.dram_tensor("out", [N_HALF, OUT], F32, kind="ExternalOutput").ap()

    with tile.TileContext(nc) as tc:
        with tc.tile_pool(name="pers", bufs=1) as pers:
            uT = pers.tile([P, 2, N_HALF], F32R)
            ident = pers.tile([P, P], F32)
            make_identity(nc, ident[:])
            wq_sb = pers.tile([P, 4, C], F32)
            wkT = pers.tile([P, 2, HD], F32)
            wvT = pers.tile([P, 2, HD], F32)
            woT = pers.tile([P, 4, OUT], F32)
            bo_bc = pers.tile([P, OUT], F32)
            weff = pers.tile([P, 2, OUT], F32R)
            ones_col = pers.tile([P, 1], F32)
            nc.vector.memset(ones_col[:], 1.0)
            ones_row = pers.tile([1, P], F32)
            nc.vector.memset(ones_row[:], 1.0)

            with (
                tc.tile_pool(name="wstage", bufs=2) as wst,
                tc.tile_pool(name="wps", bufs=2, space="PSUM") as wps,
                tc.tile_pool(name="upool", bufs=6) as upool,
                tc.tile_pool(name="pacc", bufs=1, space="PSUM") as pacc,
                tc.tile_pool(name="ptr", bufs=3, space="PSUM") as ptr,
            ):
                nc.sync.dma_start(
                    wq_sb[:], wq_d.rearrange("(a p) c -> p a c", p=P)
                )
                for w_d, wT_t in ((wk_d, wkT), (wv_d, wvT)):
                    wnat = wst.tile([P, 4, C], F32, tag="wnat", name="wnat")
                    nc.sync.dma_start(
                        wnat[:], w_d.rearrange("(a p) c -> p a c", p=P)
                    )
                    for a in range(4):
                        for b2 in range(2):
                            pst = wps.tile([P, P], F32, tag="wt", name="pst")
                            nc.tensor.transpose(
                                pst[:], wnat[:, a, b2 * P:(b2 + 1) * P], ident[:]
                            )
                            nc.any.tensor_copy(
                                wT_t[:, b2, a * P:(a + 1) * P], pst[:]
                            )
                wonat = wst.tile([P, 2, HD], F32, tag="wonat", name="wonat")
                nc.sync.dma_start(
                    wonat[:], wo_d.rearrange("(a p) c -> p a c", p=P)
                )
                for a in range(2):
                    for b2 in range(4):
                        pst = wps.tile([P, P], F32, tag="wt", name="pst")
                        nc.tensor.transpose(
                            pst[:], wonat[:, a, b2 * P:(b2 + 1) * P], ident[:]
                        )
                        nc.any.tensor_copy(
                            woT[:, b2, a * P:(a + 1) * P], pst[:]
                        )
                bo_sb = wst.tile([1, OUT], F32, tag="bo", name="bo_sb")
                nc.sync.dma_start(bo_sb[:], bo_d)
                pbo = wps.tile([P, OUT], F32, tag="wt", name="pbo")
                nc.tensor.matmul(pbo[:], ones_row[:], bo_sb[:], start=True, stop=True)
                nc.any.tensor_copy(bo_bc[:], pbo[:])

                cps = [
                    pacc.tile([P, C + 1], F32, tag=f"c{t}", name=f"c{t}")
                    for t in range(2)
                ]
                for ch in range(N_CHUNKS):
                    u1 = upool.tile([P, 4, C + 1], F32, tag="u1", name="u1")
                    nc.sync.dma_start(
                        u1[:, :, 0:C],
                        u_d[ch * 512:(ch + 1) * 512, :].rearrange(
                            "(p j) c -> p j c", p=P
                        ),
                    )
                    nc.vector.memset(u1[:, :, C:C + 1], 1.0)
                    ubf = upool.tile([P, 4, C + 1], BF16, tag="ubf", name="ubf")
                    nc.vector.tensor_copy(ubf[:], u1[:])
                    for j in range(4):
                        for t in range(2):
                            nc.tensor.matmul(
                                cps[t][:],
                                ubf[:, j, t * P:(t + 1) * P],
                                ubf[:, j, :],
                                start=(ch == 0 and j == 0),
                                stop=(ch == N_CHUNKS - 1 and j == 3),
                            )
                    if ch < MY_CHUNKS:
                        for j in range(4):
                            g = ch * 4 + j
                            tps = ptr.tile([P, 2 * P], F32, tag="uT", name="tps")
                            for t in range(2):
                                nc.tensor.transpose(
                                    tps[:, t * P:(t + 1) * P],
                                    u1[:, j, t * P:(t + 1) * P],
                                    ident[:],
                                )
                            nc.vector.tensor_copy(
                                uT[:, :, g * P:(g + 1) * P],
                                tps[:].rearrange("p (t n) -> p t n", t=2),
                            )

                cuu = pers.tile([P, 2, C + 1], F32)
                for t in range(2):
                    nc.any.tensor_copy(cuu[:, t, :], cps[t][:])

            with tc.tile_pool(name="sm", bufs=1) as sm:
                a_k = sm.tile([P, 2, HD], F32)
                a_v = sm.tile([P, 2, HD], F32)
                with tc.tile_pool(name="psA", bufs=2, space="PSUM") as psA:
                    for wT_t, a_t in ((wkT, a_k), (wvT, a_v)):
                        for t in range(2):
                            aps = psA.tile([P, HD], F32, tag="aps", name="aps")
                            for tp in range(2):
                                nc.tensor.matmul(
                                    aps[:],
                                    cuu[:, tp, t * P:(t + 1) * P],
                                    wT_t[:, tp, :],
                                    start=(tp == 0),
                                    stop=(tp == 1),
                                )
                            nc.any.tensor_copy(a_t[:, t, :], aps[:])

                mk = sm.tile([1, HD], F32)
                mv = sm.tile([1, HD], F32)
                ekk = sm.tile([1, HD], F32)
                evv = sm.tile([1, HD], F32)
                rk = sm.tile([1, HD], F32)
                rv = sm.tile([1, HD], F32)
                mkr = sm.tile([1, HD], F32)
                mvr = sm.tile([1, HD], F32)
                tmp_row = sm.tile([1, HD], F32)
                eps_row = sm.tile([1, HD], F32)
                nc.vector.memset(eps_row[:], EPS)
                with tc.tile_pool(name="psS", bufs=2, space="PSUM") as psS:
                    for wT_t, m_t in ((wkT, mk), (wvT, mv)):
                        sps = psS.tile([1, HD], F32, tag="st", name="sps")
                        for tp in range(2):
                            nc.tensor.matmul(
                                sps[:],
                                cuu[:, tp, C:C + 1],
                                wT_t[:, tp, :],
                                start=(tp == 0),
                                stop=(tp == 1),
                            )
                        nc.scalar.activation(m_t[:], sps[:], AF.Copy, scale=INV_N)
                    for t in range(2):
                        nc.vector.tensor_mul(
                            wkT[:, t, :], wkT[:, t, :], a_k[:, t, :]
                        )
                        nc.vector.tensor_mul(
                            a_v[:, t, :], a_v[:, t, :], wvT[:, t, :]
                        )
                    for m_src, e_t in ((wkT, ekk), (a_v, evv)):
                        sps = psS.tile([1, HD], F32, tag="st", name="sps")
                        for tp in range(2):
                            nc.tensor.matmul(
                                sps[:],
                                ones_col[:],
                                m_src[:, tp, :],
                                start=(tp == 0),
                                stop=(tp == 1),
                            )
                        nc.scalar.activation(e_t[:], sps[:], AF.Copy, scale=INV_N)
                for m_t, e_t, r_t in ((mk, ekk, rk), (mv, evv, rv)):
                    nc.vector.tensor_mul(tmp_row[:], m_t[:], m_t[:])
                    nc.vector.tensor_sub(tmp_row[:], e_t[:], tmp_row[:])
                    nc.vector.tensor_add(tmp_row[:], tmp_row[:], eps_row[:])
                    nc.scalar.activation(r_t[:], tmp_row[:], AF.Sqrt)
                    nc.vector.reciprocal(r_t[:], r_t[:])
                nc.vector.tensor_mul(mkr[:], mk[:], rk[:])
                nc.vector.tensor_mul(mvr[:], mv[:], rv[:])

                rk_bc = sm.tile([P, HD], F32)
                rv_bc = sm.tile([P, HD], F32)
                with tc.tile_pool(name="psB", bufs=2, space="PSUM") as psB:
                    for r_t, bc_t in ((rk, rk_bc), (rv, rv_bc)):
                        bps = psB.tile([P, HD], F32, tag="bc", name="bps")
                        nc.tensor.matmul(
                            bps[:], ones_row[:], r_t[:], start=True, stop=True
                        )
                        nc.any.tensor_copy(bc_t[:], bps[:])
                for t in range(2):
                    nc.vector.tensor_mul(a_k[:, t, :], a_k[:, t, :], rk_bc[:])
                    nc.vector.tensor_mul(wvT[:, t, :], wvT[:, t, :], rv_bc[:])

                with (
                    tc.tile_pool(name="psP", bufs=2, space="PSUM") as psP,
                    tc.tile_pool(name="psW", bufs=1, space="PSUM") as psW,
                ):
                    wps2 = [
                        psW.tile([P, OUT], F32, tag=f"weff{t}", name=f"wps{t}")
                        for t in range(2)
                    ]
                    for jp in range(4):
                        sl = slice(jp * P, (jp + 1) * P)
                        sd = psP.tile([P, P], F32, tag="sd", name="sd")
                        for tp in range(2):
                            nc.tensor.matmul(
                                sd[:],
                                wvT[:, tp, sl],
                                a_k[:, tp, sl],
                                start=(tp == 0),
                                stop=(tp == 1),
                            )
                        outr = psP.tile([P, P], F32, tag="outr", name="outr")
                        nc.tensor.matmul(
                            outr[:], mvr[:, sl], mkr[:, sl], start=True, stop=True
                        )
                        kvp = sm.tile([P, P], F32, tag=f"kv{jp}", name=f"kv{jp}")
                        nc.vector.memset(kvp[:], 0.0)
                        for g in range(2):
                            gs = slice(g * 64, g * 64 + 64)
                            nc.scalar.mul(kvp[gs, gs], sd[gs, gs], INV_N)
                            nc.vector.tensor_sub(
                                kvp[gs, gs], kvp[gs, gs], outr[gs, gs]
                            )
                        bps2 = psP.tile([P, OUT], F32, tag="bps2", name="bps2")
                        nc.tensor.matmul(
                            bps2[:],
                            kvp[:],
                            woT[:, jp, :],
                            start=True,
                            stop=True,
                        )
                        bsb = sm.tile([P, OUT], F32, tag="bsb", name="bsb")
                        nc.any.tensor_copy(bsb[:], bps2[:])
                        for t in range(2):
                            nc.tensor.matmul(
                                wps2[t][:],
                                wq_sb[:, jp, t * P:(t + 1) * P],
                                bsb[:],
                                start=(jp == 0),
                                stop=(jp == 3),
                            )
                    for t in range(2):
                        nc.any.tensor_copy(weff[:, t, :], wps2[t][:])

            with (
                tc.tile_pool(name="opool", bufs=4) as opool,
                tc.tile_pool(name="pout", bufs=4, space="PSUM") as pout,
            ):
                for ch in range(MY_CHUNKS):
                    osb = opool.tile([P, 4, OUT], F32, tag="osb", name="osb")
                    for j in range(4):
                        g = ch * 4 + j
                        ops = pout.tile([P, OUT], F32, tag="ops", name="ops")
                        for t in range(2):
                            nc.tensor.matmul(
                                ops[:],
                                uT[:, t, g * P:(g + 1) * P],
                                weff[:, t, :],
                                start=(t == 0),
                                stop=(t == 1),
                            )
                        nc.vector.tensor_add(osb[:, j, :], ops[:], bo_bc[:])
                    nc.sync.dma_start(
                        out_d[ch * 512:(ch + 1) * 512, :].rearrange(
                            "(p j) c -> p j c", p=P
                        ),
                        osb[:],
                    )

    nc.compile()
    return nc


_NC_CACHE = None


def _get_nc():
    global _NC_CACHE
    if _NC_CACHE is None:
        _NC_CACHE = build_nc()
    return _NC_CACHE


def make_in_maps(u_src, Wq, Wk, Wv, Wo, bo):
    in_maps = []
    for c in range(8):
        b, half = c // 2, c % 2
        ub = u_src[b]
        mine = ub[half * N_HALF:(half + 1) * N_HALF]
        other = ub[(1 - half) * N_HALF:(2 - half) * N_HALF]
        u_perm = np.ascontiguousarray(np.concatenate([mine, other], axis=0))
        in_maps.append(
            {
                "u": u_perm,
                "wq": np.ascontiguousarray(Wq),
                "wk": np.ascontiguousarray(Wk),
                "wv": np.ascontiguousarray(Wv),
                "wo": np.ascontiguousarray(Wo),
                "bo": np.ascontiguousarray(bo.reshape(1, OUT)),
            }
        )
    return in_maps


def assemble_output(results):
    out = np.empty((4, N_FULL, OUT), dtype=np.float32)
    for c in range(8):
        b, half = c // 2, c % 2
        out[b, half * N_HALF:(half + 1) * N_HALF] = results[c]["out"]
    return out


def run(inputs, trace=False, tmpdir=None):
    u_src = np.asarray(inputs["u_src"], dtype=np.float32)
    Wq = np.asarray(inputs["Wq"], dtype=np.float32)
    Wk = np.asarray(inputs["Wk"], dtype=np.float32)
    Wv = np.asarray(inputs["Wv"], dtype=np.float32)
    Wo = np.asarray(inputs["Wo"], dtype=np.float32)
    bo = np.asarray(inputs["bo"], dtype=np.float32)
    nc = _get_nc()
    in_maps = make_in_maps(u_src, Wq, Wk, Wv, Wo, bo)
    res = run_bass_kernel_spmd(
        nc, in_maps, core_ids=list(range(8)), trace=trace, tmpdir=tmpdir
    )
    return assemble_output(res.results), res


def kernel(**inputs):
    out, _ = run(inputs, trace=False)
    return out


# revision 12
# speedup vs baseline: 1.2893x; 1.2893x over previous
import numpy as np

import concourse.bass as bass
import concourse.tile as tile
from concourse import bacc, mybir
from concourse.bass_utils import run_bass_kernel_spmd
from concourse.masks import make_identity

F32 = mybir.dt.float32
BF16 = mybir.dt.bfloat16
AL = mybir.AluOpType
AF = mybir.ActivationFunctionType

P = 128
N_FULL = 16384
N_HALF = 8192
C = 256
HD = 512
OUT = 256
EPS = 1e-5
N_CHUNKS = 32
MY_CHUNKS = 16
INV_N = 1.0 / float(N_FULL)


def build_nc():
    nc = bacc.Bacc(
        "TRN2",
        target_bir_lowering=False,
        debug=False,
        num_devices=8,
    )
    u_d = nc.dram_tensor("u", [N_FULL, C], F32, kind="ExternalInput").ap()
    wq_d = nc.dram_tensor("wq", [HD, C], F32, kind="ExternalInput").ap()
    wk_d = nc.dram_tensor("wk", [HD, C], F32, kind="ExternalInput").ap()
    wv_d = nc.dram_tensor("wv", [HD, C], F32, kind="ExternalInput").ap()
    wo_d = nc.dram_tensor("wo", [OUT, HD], F32, kind="ExternalInput").ap()
    out_d = nc.dram_tensor("out", [N_HALF, OUT], F32, kind="ExternalOutput").ap()

    with tile.TileContext(nc) as tc:
        with tc.tile_pool(name="pers", bufs=1) as pers:
            uT = pers.tile([P, 2, N_HALF], BF16)
            ident = pers.tile([P, P], F32)
            make_identity(nc, ident[:])
            ident_bf = pers.tile([P, P], BF16)
            nc.vector.tensor_copy(ident_bf[:], ident[:])
            wq_sb = pers.tile([P, 4, C], F32)
            wkT = pers.tile([P, 2, HD], F32)
            wvT = pers.tile([P, 2, HD], F32)
            woT = pers.tile([P, 4, OUT], F32)
            weff = pers.tile([P, 2, OUT], BF16)
            ones_col = pers.tile([P, 1], F32)
            nc.vector.memset(ones_col[:], 1.0)
            ones_row = pers.tile([1, P], F32)
            nc.vector.memset(ones_row[:], 1.0)
            one1 = pers.tile([1, 1], F32)
            nc.vector.memset(one1[:], 1.0)
            warm = pers.tile([1, 8], F32)
            nc.vector.memset(warm[:], 1.0)
            nc.scalar.mul(warm[:], warm[:], 1.0)
            nc.scalar.activation(warm[:], warm[:], AF.Sqrt)

            with (
                tc.tile_pool(name="wstage", bufs=2) as wst,
                tc.tile_pool(name="wps", bufs=2, space="PSUM") as wps,
                tc.tile_pool(name="upool", bufs=5) as upool,
                tc.tile_pool(name="pacc", bufs=1, space="PSUM") as pacc,
                tc.tile_pool(name="ptr", bufs=3, space="PSUM") as ptr,
            ):
                nc.sync.dma_start(
                    wq_sb[:], wq_d.rearrange("(a p) c -> p a c", p=P)
                )
                for w_d, wT_t in ((wk_d, wkT), (wv_d, wvT)):
                    wnat = wst.tile([P, 4, C], F32, tag="wnat", name="wnat")
                    nc.sync.dma_start(
                        wnat[:], w_d.rearrange("(a p) c -> p a c", p=P)
                    )
                    for a in range(4):
                        for b2 in range(2):
                            pst = wps.tile([P, P], F32, tag="wt", name="pst")
                            nc.tensor.transpose(
                                pst[:], wnat[:, a, b2 * P:(b2 + 1) * P], ident[:]
                            )
                            nc.any.tensor_copy(
                                wT_t[:, b2, a * P:(a + 1) * P], pst[:]
                            )
                wonat = wst.tile([P, 2, HD], F32, tag="wonat", name="wonat")
                nc.sync.dma_start(
                    wonat[:], wo_d.rearrange("(a p) c -> p a c", p=P)
                )
                for a in range(2):
                    for b2 in range(4):
                        pst = wps.tile([P, P], F32, tag="wt", name="pst")
                        nc.tensor.transpose(
                            pst[:], wonat[:, a, b2 * P:(b2 + 1) * P], ident[:]
                        )
                        nc.any.tensor_copy(
                            woT[:, b2, a * P:(a + 1) * P], pst[:]
                        )

                cps = [
                    pacc.tile([P, C + 1], F32, tag=f"c{t}", name=f"c{t}")
                    for t in range(2)
                ]
                for ch in range(N_CHUNKS):
                    u1 = upool.tile([P, 4, C + 1], F32, tag="u1", name="u1")
                    nc.sync.dma_start(
                        u1[:, :, 0:C],
                        u_d[ch * 512:(ch + 1) * 512, :].rearrange(
                            "(p j) c -> p j c", p=P
                        ),
                    )
                    nc.vector.memset(u1[:, :, C:C + 1], 1.0)
                    ubf = upool.tile([P, 4, C + 1], BF16, tag="ubf", name="ubf")
                    nc.vector.tensor_copy(ubf[:], u1[:])
                    for j in range(4):
                        for t in range(2):
                            nc.tensor.matmul(
                                cps[t][:],
                                ubf[:, j, t * P:(t + 1) * P],
                                ubf[:, j, :],
                                start=(ch == 0 and j == 0),
                                stop=(ch == N_CHUNKS - 1 and j == 3),
                            )
                    if ch < MY_CHUNKS:
                        for j in range(4):
                            g = ch * 4 + j
                            tps = ptr.tile([P, 2 * P], BF16, tag="uT", name="tps")
                            for t in range(2):
                                nc.tensor.transpose(
                                    tps[:, t * P:(t + 1) * P],
                                    ubf[:, j, t * P:(t + 1) * P],
                                    ident_bf[:],
                                )
                            nc.vector.tensor_copy(
                                uT[:, :, g * P:(g + 1) * P],
                                tps[:].rearrange("p (t n) -> p t n", t=2),
                            )

                cuu = pers.tile([P, 2, C + 1], F32)
                for t in range(2):
                    nc.any.tensor_copy(cuu[:, t, :], cps[t][:])

            with tc.tile_pool(name="sm", bufs=1) as sm:
                psA_ctx = tc.tile_pool(name="psA", bufs=1, space="PSUM")
                psA = psA_ctx.__enter__()
                a_k = sm.tile([P, 2, HD], F32)
                a_v = sm.tile([P, 2, HD], F32)
                for wT_t, a_t in ((wkT, a_k), (wvT, a_v)):
                    for t in range(2):
                        aps = psA.tile([P, HD], F32, tag="aps", bufs=2, name="aps")
                        for tp in range(2):
                            nc.tensor.matmul(
                                aps[:],
                                cuu[:, tp, t * P:(t + 1) * P],
                                wT_t[:, tp, :],
                                start=(tp == 0),
                                stop=(tp == 1),
                            )
                        nc.any.tensor_copy(a_t[:, t, :], aps[:])

                mk = sm.tile([1, HD], F32)
                mv = sm.tile([1, HD], F32)
                ekk = sm.tile([1, HD], F32)
                evv = sm.tile([1, HD], F32)
                tk = sm.tile([1, HD], F32)
                tv = sm.tile([1, HD], F32)
                vark = sm.tile([1, HD], F32)
                varv = sm.tile([1, HD], F32)
                for wT_t, m_t in ((wkT, mk), (wvT, mv)):
                    sps = psA.tile([1, HD], F32, tag="st", bufs=2, name="sps")
                    for tp in range(2):
                        nc.tensor.matmul(
                            sps[:],
                            cuu[:, tp, C:C + 1],
                            wT_t[:, tp, :],
                            start=(tp == 0),
                            stop=(tp == 1),
                        )
                    nc.scalar.activation(m_t[:], sps[:], AF.Copy, scale=INV_N)
                for t in range(2):
                    nc.vector.tensor_mul(wkT[:, t, :], wkT[:, t, :], a_k[:, t, :])
                    nc.vector.tensor_mul(a_v[:, t, :], a_v[:, t, :], wvT[:, t, :])
                for m_src, e_t in ((wkT, ekk), (a_v, evv)):
                    sps = psA.tile([1, HD], F32, tag="st", bufs=2, name="sps")
                    for tp in range(2):
                        nc.tensor.matmul(
                            sps[:],
                            ones_col[:],
                            m_src[:, tp, :],
                            start=(tp == 0),
                            stop=(tp == 1),
                        )
                    nc.scalar.activation(e_t[:], sps[:], AF.Copy, scale=INV_N)
                nc.vector.tensor_mul(tk[:], mk[:], mk[:])
                nc.vector.tensor_mul(tv[:], mv[:], mv[:])
                nc.vector.tensor_sub(vark[:], ekk[:], tk[:])
                nc.vector.tensor_sub(varv[:], evv[:], tv[:])

                eps_col = sm.tile([P, 4], F32)
                nc.vector.memset(eps_col[:], EPS)
                rk_col = sm.tile([P, 4], F32)
                rv_col = sm.tile([P, 4], F32)
                for var_row, r_col in ((vark, rk_col), (varv, rv_col)):
                    vc = psA.tile([P, 4], F32, tag="vc", bufs=2, name="vc")
                    for g in range(4):
                        nc.tensor.matmul(
                            vc[:, g:g + 1],
                            var_row[0:1, g * P:(g + 1) * P],
                            one1[:],
                            start=True,
                            stop=True,
                        )
                    nc.vector.tensor_add(r_col[:], vc[:], eps_col[:])
                    nc.scalar.activation(r_col[:], r_col[:], AF.Sqrt)
                    nc.vector.reciprocal(r_col[:], r_col[:])
                rk_row = sm.tile([1, HD], F32)
                rk_bc = sm.tile([P, HD], F32)
                rps = psA.tile([1, HD], F32, tag="st", bufs=2, name="rps")
                for g in range(4):
                    nc.tensor.matmul(
                        rps[0:1, g * P:(g + 1) * P],
                        rk_col[:, g:g + 1],
                        ident[:],
                        start=True,
                        stop=True,
                    )
                nc.scalar.mul(rk_row[:], rps[:], 1.0)
                bps = psA.tile([P, HD], F32, tag="aps", bufs=2, name="bps")
                nc.tensor.matmul(bps[:], ones_row[:], rk_row[:], start=True, stop=True)
                nc.any.tensor_copy(rk_bc[:], bps[:])

                psA_ctx.__exit__(None, None, None)
                with tc.tile_pool(name="psP", bufs=1, space="PSUM") as psP:
                    wps2 = [
                        psP.tile([P, OUT], F32, tag=f"weff{t}", name=f"wps{t}")
                        for t in range(2)
                    ]
                    for jp in range(4):
                        sl = slice(jp * P, (jp + 1) * P)
                        sd = psP.tile([P, P], F32, tag="sd", bufs=2, name="sd")
                        for tp in range(2):
                            nc.tensor.matmul(
                                sd[:],
                                wvT[:, tp, sl],
                                a_k[:, tp, sl],
                                start=(tp == 0),
                                stop=(tp == 1),
                            )
                        outr = psP.tile([P, P], F32, tag="outr", bufs=2, name="outr")
                        nc.tensor.matmul(
                            outr[:], mv[:, sl], mk[:, sl], start=True, stop=True
                        )
                        kvp = sm.tile([P, P], F32, tag=f"kv{jp}", name=f"kv{jp}")
                        nc.vector.memset(kvp[:], 0.0)
                        for g in range(2):
                            gs = slice(g * 64, g * 64 + 64)
                            nc.scalar.mul(kvp[gs, gs], sd[gs, gs], INV_N)
                            nc.vector.tensor_sub(
                                kvp[gs, gs], kvp[gs, gs], outr[gs, gs]
                            )
                        nc.vector.tensor_mul(kvp[:], kvp[:], rk_bc[:, sl])
                        nc.vector.tensor_scalar_mul(kvp[:], kvp[:], rv_col[:, jp:jp + 1])
                        bps2 = psP.tile([P, OUT], F32, tag="bps2", bufs=2, name="bps2")
                        nc.tensor.matmul(
                            bps2[:], kvp[:], woT[:, jp, :], start=True, stop=True
                        )
                        bsb = sm.tile([P, OUT], F32, tag="bsb", name="bsb")
                        nc.any.tensor_copy(bsb[:], bps2[:])
                        for t in range(2):
                            nc.tensor.matmul(
                                wps2[t][:],
                                wq_sb[:, jp, t * P:(t + 1) * P],
                                bsb[:],
                                start=(jp == 0),
                                stop=(jp == 3),
                            )
                    for t in range(2):
                        nc.any.tensor_copy(weff[:, t, :], wps2[t][:])

            with (
                tc.tile_pool(name="opool", bufs=4) as opool,
                tc.tile_pool(name="pout", bufs=4, space="PSUM") as pout,
            ):
                for ch in range(MY_CHUNKS):
                    osb = opool.tile([P, 4, OUT], F32, tag="osb", name="osb")
                    for j in range(4):
                        g = ch * 4 + j
                        ops = pout.tile([P, OUT], F32, tag="ops", name="ops")
                        for t in range(2):
                            nc.tensor.matmul(
                                ops[:],
                                uT[:, t, g * P:(g + 1) * P],
                                weff[:, t, :],
                                start=(t == 0),
                                stop=(t == 1),
                            )
                        nc.vector.tensor_copy(osb[:, j, :], ops[:])
                    nc.sync.dma_start(
                        out_d[ch * 512:(ch + 1) * 512, :].rearrange(
                            "(p j) c -> p j c", p=P
                        ),
                        osb[:],
                    )

    nc.compile()
    return nc


_NC_CACHE = None


def _get_nc():
    global _NC_CACHE
    if _NC_CACHE is None:
        _NC_CACHE = build_nc()
    return _NC_CACHE


def make_in_maps(u_src, Wq, Wk, Wv, Wo):
    in_maps = []
    for c in range(8):
        b, half = c // 2, c % 2
        ub = u_src[b]
        mine = ub[half * N_HALF:(half + 1) * N_HALF]
        other = ub[(1 - half) * N_HALF:(2 - half) * N_HALF]
        u_perm = np.ascontiguousarray(np.concatenate([mine, other], axis=0))
        in_maps.append(
            {
                "u": u_perm,
                "wq": np.ascontiguousarray(Wq),
                "wk": np.ascontiguousarray(Wk),
                "wv": np.ascontiguousarray(Wv),
                "wo": np.ascontiguousarray(Wo),
            }
        )
    return in_maps


def assemble_output(results, bo):
    out = np.empty((4, N_FULL, OUT), dtype=np.float32)
    for c in range(8):
        b, half = c // 2, c % 2
        out[b, half * N_HALF:(half + 1) * N_HALF] = results[c]["out"]
    if np.any(bo):
        out += bo.reshape(1, 1, OUT)
    return out


def run(inputs, trace=False, tmpdir=None):
    u_src = np.asarray(inputs["u_src"], dtype=np.float32)
    Wq = np.asarray(inputs["Wq"], dtype=np.float32)
    Wk = np.asarray(inputs["Wk"], dtype=np.float32)
    Wv = np.asarray(inputs["Wv"], dtype=np.float32)
    Wo = np.asarray(inputs["Wo"], dtype=np.float32)
    bo = np.asarray(inputs["bo"], dtype=np.float32)
    nc = _get_nc()
    in_maps = make_in_maps(u_src, Wq, Wk, Wv, Wo)
    res = run_bass_kernel_spmd(
        nc, in_maps, core_ids=list(range(8)), trace=trace, tmpdir=tmpdir
    )
    return assemble_output(res.results, bo), res


def kernel(**inputs):
    out, _ = run(inputs, trace=False)
    return out
